# revision 2
# baseline (speedup 1.0000x reference)
"""Multi-head causal self-attention (B=1, S=4096, D=1024, H=16) on 8 TRN2
NeuronCores, tensor-parallel over heads (2 heads per core).

v2: all-bf16 dataflow engineered to the TimelineSim cost model.
  - qkv^T = (X @ W)^T via matmul(lhsT=W_tile, rhs=X^T tile); K needs no bias
    (constant-in-t score shifts cancel in softmax), V's bias is folded into
    bo on the host (bo' = bo + bv @ Wo), so only Q gets a bias add.
  - V is produced directly in [t, d] layout by a second matmul pass with
    X^T tiles as the stationary operand (lhsT=xt[:,t-tile], rhs=Wv k-tile),
    eliminating all on-device transposes.
  - scores^T [t, s] per head via matmul(lhsT=K^T tile, rhs=Q^T chunk), full
    diagonal trim (bf16 matmuls have no free-dim>=256 constraint).
  - softmax without max-subtraction; exp on ACT with 1/8 scale folded in;
    pt in bf16 so the diagonal mask multiply runs in DVE 2x mode.
  - P^T @ V via matmul(lhsT=vhat[t,d]+ones column, rhs=pt) -> numerator rows
    0-63 and denominator on row 64 of PSUM.
  - denominator reciprocal broadcast back via a K=1 ones matmul; divide on
    DVE; y^T partial = Wo^T @ out^T; bf16 partials DMAed out once per chunk;
    host sums the 8 partials, adds bo', transposes.
  - one fused ~1MB DMA per chunk each way; proj/qkv/V matmuls of neighboring
    chunks are interleaved into the attention group loop as PE filler so the
    PE never idles behind the ACT-bound exp cadence.
"""

import sys

sys.path.insert(0, "/opt/trn_rl_repo")

import functools
import numpy as np
import ml_dtypes

D = 1024
H = 16
HD = 64
NCORES = 8
HPC = H // NCORES  # heads per core = 2
P = 128
CH = 512  # s-chunk width
BF16 = ml_dtypes.bfloat16


def build_nc(S):
    import concourse.bacc as bacc
    import concourse.mybir as mybir
    from concourse import tile

    f32 = mybir.dt.float32
    f32r = mybir.dt.float32r
    bf16 = mybir.dt.bfloat16
    ADD = mybir.AluOpType.add
    EXP = mybir.ActivationFunctionType.Exp

    NCHUNK = S // CH
    NT = S // P  # number of 128-row t-tiles
    ND = D // P  # 8 d-tiles

    nc = bacc.Bacc("TRN2", target_bir_lowering=False, debug=False)

    xt_d = nc.dram_tensor("xt", [D, S], bf16, kind="ExternalInput")
    wqk_d = nc.dram_tensor("wqk", [D, 2 * HPC * HD], bf16, kind="ExternalInput")
    wv_d = nc.dram_tensor("wv", [D, HPC * HD], bf16, kind="ExternalInput")
    bq_d = nc.dram_tensor("bq", [HPC * HD], f32, kind="ExternalInput")
    wo_d = nc.dram_tensor("wo", [HPC * HD, D], bf16, kind="ExternalInput")
    masks_d = nc.dram_tensor("masks", [P, 4, CH], bf16, kind="ExternalInput")
    ones_d = nc.dram_tensor("ones", [1, HD], f32r, kind="ExternalInput")
    yt_d = nc.dram_tensor("yt", [D, S], bf16, kind="ExternalOutput")

    # chunk-granularity DRAM views: partition-major [p, dtile, s]
    xt_v = xt_d[:].rearrange("(dt p) s -> p dt s", p=P)
    yt_v = yt_d[:].rearrange("(dt p) s -> p dt s", p=P)

    with tile.TileContext(nc) as tc:
        with (
            tc.tile_pool(name="consts", bufs=1) as consts,
            tc.tile_pool(name="xtp", bufs=3) as xtp,
            tc.tile_pool(name="ptp", bufs=4) as ptp,
            tc.tile_pool(name="nmp", bufs=4) as nmp,
            tc.tile_pool(name="otp", bufs=2) as otp,
            tc.tile_pool(name="rcp", bufs=4) as rcp,
            tc.tile_pool(name="ytp", bufs=2) as ytp,
            tc.tile_pool(name="scp", bufs=2, space="PSUM") as scp,
            tc.tile_pool(name="avp", bufs=2, space="PSUM") as avp,
            tc.tile_pool(name="qyp", bufs=2, space="PSUM") as qyp,
        ):
            # ---- persistent SBUF ----
            wqk_sb = consts.tile([P, ND, 2 * HPC * HD], bf16)
            wv_sb = consts.tile([P, ND, HPC * HD], bf16)
            bq_sb = consts.tile([P, 1], f32)
            wo_sb = consts.tile([HPC * HD, D], bf16)
            masks_sb = consts.tile([P, 4, CH], bf16)
            ones_sb = consts.tile([P, HD], f32r)
            qt_sb = consts.tile([P, S], bf16)  # Q^T: h0 parts 0-63, h1 64-127
            kt_sb = consts.tile([P, S], bf16)
            # V-hat per head: [t-part, NT tiles, 72] (cols 0-63 = V, 64 = ones)
            vhat = [
                consts.tile([P, NT, 72], bf16, tag=f"vhat{h}", name=f"vhat{h}")
                for h in range(HPC)
            ]

            nc.sync.dma_start(bq_sb[:], bq_d[:].rearrange("(i p) -> p i", p=P))
            for h in range(HPC):
                nc.vector.memset(vhat[h][:, :, 64:65], 1.0)

            xts = {}  # chunk j -> xt tile

            def load_x(j, split):
                xt_t = xtp.tile([P, ND, CH], bf16, tag="xt", name="xt_t")
                if split:  # per-dtile loads so first matmuls start early
                    for d in range(ND):
                        nc.sync.dma_start(
                            wqk_sb[:, d, :],
                            wqk_d[d * P : (d + 1) * P, :],
                        )
                        nc.sync.dma_start(
                            xt_t[:, d, :], xt_v[:, d, j * CH : (j + 1) * CH]
                        )
                else:
                    nc.sync.dma_start(xt_t[:], xt_v[:, :, j * CH : (j + 1) * CH])
                xts[j] = xt_t

            def emit_qkv_c(j, c):
                """Q (c=0) or K (c=1) projection for s-chunk j: one psum tile."""
                xt_t = xts[j]
                ps = qyp.tile([P, CH], f32, tag="qy", name=f"qkps{c}")
                for d in range(ND):
                    nc.tensor.matmul(
                        ps[:],
                        wqk_sb[:, d, c * P : (c + 1) * P],
                        xt_t[:, d, :],
                        start=(d == 0),
                        stop=(d == ND - 1),
                    )
                if c == 0:
                    nc.vector.tensor_scalar(
                        out=qt_sb[:, j * CH : (j + 1) * CH],
                        in0=ps[:],
                        scalar1=bq_sb[:, 0:1],
                        scalar2=None,
                        op0=ADD,
                    )
                else:
                    nc.vector.tensor_copy(kt_sb[:, j * CH : (j + 1) * CH], ps[:])

            def emit_v(j, sub):
                """V[t, d] for 128-t subtile `sub` of chunk j, both heads."""
                xt_t = xts[j]
                vps = qyp.tile([P, P], f32, tag="qy", name="vps")
                for d in range(ND):
                    nc.tensor.matmul(
                        vps[:],
                        xt_t[:, d, sub * P : (sub + 1) * P],
                        wv_sb[:, d, :],
                        start=(d == 0),
                        stop=(d == ND - 1),
                    )
                tt = 4 * j + sub
                for h in range(HPC):
                    nc.vector.tensor_copy(
                        vhat[h][:, tt, 0:64], vps[:, 64 * h : 64 * h + 64]
                    )

            def emit_proj(j, ot, e):
                """output projection for chunk j, d-tile e."""
                yt_ps = qyp.tile([P, CH], f32, tag="qy", name="ytps")
                nc.tensor.matmul(
                    yt_ps[:],
                    wo_sb[:, e * P : (e + 1) * P],
                    ot[:],
                    start=True,
                    stop=True,
                )
                yt_st = yt_stage[j % 2]
                nc.vector.tensor_copy(yt_st[:, e, :], yt_ps[:])
                if e == ND - 1:
                    nc.sync.dma_start(yt_v[:, :, j * CH : (j + 1) * CH], yt_st[:])

            yt_stage = [
                ytp.tile([P, ND, CH], bf16, tag="yt", name=f"ytst{i}")
                for i in range(2)
            ]

            # ---- prologue: chunk 0 with interleaved weight loads ----
            load_x(0, split=True)
            emit_qkv_c(0, 0)
            nc.sync.dma_start(wv_sb[:], wv_d[:].rearrange("(dt p) c -> p dt c", p=P))
            emit_qkv_c(0, 1)
            nc.sync.dma_start(masks_sb[:], masks_d[:])
            nc.sync.dma_start(ones_sb[64:65, :], ones_d[:])
            nc.sync.dma_start(wo_sb[:], wo_d[:])
            load_x(1, split=False)
            for sub in range(4):
                emit_v(0, sub)
            emit_qkv_c(1, 0)
            emit_qkv_c(1, 1)
            for sub in range(4):
                emit_v(1, sub)

            # division state carried across chunks: (rcs, nms) per head
            carried = None  # (j_prev, rcs, nms)

            for j in range(NCHUNK):
                ntt = 4 * (j + 1)
                av = [
                    avp.tile([P, CH], f32, tag="av", name=f"av{h}")
                    for h in range(HPC)
                ]

                # ---- build filler list: PE work to interleave into groups ----
                fillers = []
                if carried is not None:
                    jp, rcs, nms = carried

                    def div_and_proj(jp=jp, rcs=rcs, nms=nms):
                        ot = otp.tile([P, CH], bf16, tag="ot", name="ot")
                        for h in range(HPC):
                            bc = qyp.tile([HD, CH], f32, tag="qy", name="bc")
                            nc.tensor.matmul(
                                bc[:],
                                ones_sb[64:65, 0:HD],
                                rcs[h][64:65, :],
                                start=True,
                                stop=True,
                            )
                            nc.vector.tensor_mul(
                                ot[64 * h : 64 * h + 64, :], nms[h][:], bc[:]
                            )
                        return ot

                    ot_box = []

                    def mk_div():
                        ot_box.append(div_and_proj())

                    fillers.append(mk_div)
                    for e in range(ND):
                        fillers.append(
                            lambda jp=jp, e=e: emit_proj(jp, ot_box[0], e)
                        )
                if j + 2 < NCHUNK:
                    jf = j + 2
                    fillers.append(lambda jf=jf: load_x(jf, split=False))
                    fillers.append(lambda jf=jf: emit_qkv_c(jf, 0))
                    fillers.append(lambda jf=jf: emit_qkv_c(jf, 1))
                    for sub in range(4):
                        fillers.append(lambda jf=jf, sub=sub: emit_v(jf, sub))

                # spread fillers over the group loop
                nfill = len(fillers)
                fill_every = max(1, ntt // max(nfill, 1))
                fill_i = 0

                def soff(tt):
                    o = (tt - 4 * j) * P if tt >= 4 * j else 0
                    return min(max(0, o), 3 * P)

                def flush(tt, sc):
                    """exp + mask + AV for t-tile tt (both heads)."""
                    o = soff(tt)
                    pt = ptp.tile([P, HPC, CH], bf16, tag="pt", name="pt")
                    sc_v = sc[:].rearrange("p (g c) -> p g c", c=CH)
                    nc.scalar.activation(
                        pt[:, :, o:], sc_v[:, :, o:], EXP, scale=0.125
                    )
                    if tt >= 4 * j:  # diagonal: one masked mul for both heads
                        k = tt - 4 * j
                        nc.vector.tensor_mul(
                            pt[:, :, o:],
                            pt[:, :, o:],
                            masks_sb[:, k : k + 1, o:].broadcast_to(
                                [P, HPC, CH - o]
                            ),
                        )
                    for h in range(HPC):
                        nc.tensor.matmul(
                            av[h][0:65, o:],
                            vhat[h][:, tt, 0:65],
                            pt[:, h, o:],
                            start=(tt == 0),
                            stop=(tt == ntt - 1),
                        )

                pending = None
                for tt in range(ntt):
                    o = soff(tt)
                    sc = scp.tile([P, HPC * CH], f32, tag="sc", name="sc")
                    for h in range(HPC):
                        nc.tensor.matmul(
                            sc[:, h * CH + o : (h + 1) * CH],
                            kt_sb[64 * h : 64 * h + 64, tt * P : (tt + 1) * P],
                            qt_sb[64 * h : 64 * h + 64, j * CH + o : (j + 1) * CH],
                            start=True,
                            stop=True,
                        )
                    # interleave filler PE work between score groups
                    if fill_i < nfill and (tt % fill_every == fill_every - 1):
                        fillers[fill_i]()
                        fill_i += 1
                    if pending is not None:
                        flush(*pending)
                    pending = (tt, sc)
                while fill_i < nfill:
                    fillers[fill_i]()
                    fill_i += 1
                if pending is not None:
                    flush(*pending)

                # ---- reciprocals + numerator copies (free the av tiles) ----
                rcs, nms = [], []
                for h in range(HPC):
                    rc = rcp.tile([P, CH], f32r, tag="rc", name="rc")
                    with nc.allow_low_precision("fp32r recip feeds fp22 matmul"):
                        nc.vector.reciprocal(rc[64:65, :], av[h][64:65, :])
                    nm = nmp.tile([HD, CH], f32, tag="nm", name="nm")
                    nc.vector.tensor_copy(nm[:], av[h][0:64, :])
                    rcs.append(rc)
                    nms.append(nm)
                carried = (j, rcs, nms)

            # ---- epilogue: division + projection for the last chunk ----
            jp, rcs, nms = carried
            ot = otp.tile([P, CH], bf16, tag="ot", name="ot")
            for h in range(HPC):
                bc = qyp.tile([HD, CH], f32, tag="qy", name="bc")
                nc.tensor.matmul(
                    bc[:],
                    ones_sb[64:65, 0:HD],
                    rcs[h][64:65, :],
                    start=True,
                    stop=True,
                )
                nc.vector.tensor_mul(ot[64 * h : 64 * h + 64, :], nms[h][:], bc[:])
            for e in range(ND):
                emit_proj(jp, ot, e)

    return nc


@functools.lru_cache(maxsize=2)
def _get_nc(S):
    nc = build_nc(S)
    nc.compile()
    return nc


def make_in_maps(input, Wqkv, bqkv, Wo, S):
    """Host-side shard prep. input [1,S,D] (or [S,D]); returns per-core dicts."""
    x = np.asarray(input, dtype=np.float32).reshape(S, D)
    xt = np.ascontiguousarray(x.T.astype(BF16))
    Wqkv = np.asarray(Wqkv, dtype=np.float32)
    bqkv = np.asarray(bqkv, dtype=np.float32)
    Wo = np.asarray(Wo, dtype=np.float32)

    # causal masks for the 4 diagonal 128-blocks of a 512 chunk
    pp = np.arange(P)[:, None]
    ff = np.arange(CH)[None, :]
    masks = np.stack(
        [(ff >= pp + P * k).astype(BF16) for k in range(4)], axis=1
    )  # [128, 4, 512]
    masks = np.ascontiguousarray(masks)

    Wq, Wk, Wv = Wqkv[:, 0:D], Wqkv[:, D : 2 * D], Wqkv[:, 2 * D : 3 * D]
    bq = bqkv[0:D]

    in_maps = []
    for c in range(NCORES):
        hs = [c * HPC + i for i in range(HPC)]
        cols = lambda W: np.concatenate(
            [W[:, h * HD : (h + 1) * HD] for h in hs], axis=1
        )
        colsb = lambda b: np.concatenate(
            [b[h * HD : (h + 1) * HD] for h in hs], axis=0
        )
        wqk_l = np.ascontiguousarray(
            np.concatenate([cols(Wq), cols(Wk)], axis=1).astype(BF16)
        )
        wv_l = np.ascontiguousarray(cols(Wv).astype(BF16))
        bq_l = np.ascontiguousarray(colsb(bq).astype(np.float32))
        wo_l = np.ascontiguousarray(
            Wo[hs[0] * HD : hs[0] * HD + HPC * HD, :].astype(BF16)
        )
        in_maps.append(
            {
                "xt": xt,
                "wqk": wqk_l,
                "wv": wv_l,
                "bq": bq_l,
                "wo": wo_l,
                "masks": masks,
                "ones": np.ones((1, HD), dtype=np.float32),
            }
        )
    return in_maps


def kernel(input, Wqkv, bqkv, Wo, bo):
    from concourse.bass_utils import run_bass_kernel_spmd

    S = np.asarray(input).reshape(-1, D).shape[0]
    nc = _get_nc(S)
    in_maps = make_in_maps(input, Wqkv, bqkv, Wo, S)
    res = None
    last_exc = None
    for _attempt in range(3):  # transient NRT/device errors: retry
        try:
            res = run_bass_kernel_spmd(nc, in_maps, core_ids=list(range(NCORES)))
            break
        except Exception as e:  # noqa: BLE001
            last_exc = e
    if res is None:
        raise last_exc
    yt = res.results[0]["yt"].astype(np.float32)
    for r in res.results[1:]:
        yt += r["yt"].astype(np.float32)
    # fold the V bias through the output projection: y += bv @ Wo + bo
    bv = np.asarray(bqkv, dtype=np.float32)[2 * D : 3 * D]
    bo_eff = np.asarray(bo, dtype=np.float32) + bv @ np.asarray(
        Wo, dtype=np.float32
    )
    y = yt.T + bo_eff[None, :]
    return np.ascontiguousarray(y, dtype=np.float32).reshape(1, S, D)


# revision 7
# speedup vs baseline: 1.0112x; 1.0112x over previous
"""Multi-head causal self-attention (B=1, S=4096, D=1024, H=16) on 8 TRN2
NeuronCores, tensor-parallel over heads (2 heads per core).

v2: all-bf16 dataflow engineered to the TimelineSim cost model.
  - qkv^T = (X @ W)^T via matmul(lhsT=W_tile, rhs=X^T tile); K needs no bias
    (constant-in-t score shifts cancel in softmax), V's bias is folded into
    bo on the host (bo' = bo + bv @ Wo), so only Q gets a bias add.
  - V is produced directly in [t, d] layout by a second matmul pass with
    X^T tiles as the stationary operand (lhsT=xt[:,t-tile], rhs=Wv k-tile),
    eliminating all on-device transposes.
  - scores^T [t, s] per head via matmul(lhsT=K^T tile, rhs=Q^T chunk), full
    diagonal trim (bf16 matmuls have no free-dim>=256 constraint).
  - softmax without max-subtraction; exp on ACT with 1/8 scale folded in;
    pt in bf16 so the diagonal mask multiply runs in DVE 2x mode.
  - P^T @ V via matmul(lhsT=vhat[t,d]+ones column, rhs=pt) -> numerator rows
    0-63 and denominator on row 64 of PSUM.
  - denominator reciprocal broadcast back via a K=1 ones matmul; divide on
    DVE; y^T partial = Wo^T @ out^T; bf16 partials DMAed out once per chunk;
    host sums the 8 partials, adds bo', transposes.
  - one fused ~1MB DMA per chunk each way; proj/qkv/V matmuls of neighboring
    chunks are interleaved into the attention group loop as PE filler so the
    PE never idles behind the ACT-bound exp cadence.
"""

import sys

sys.path.insert(0, "/opt/trn_rl_repo")

import functools
import numpy as np
import ml_dtypes

D = 1024
H = 16
HD = 64
NCORES = 8
HPC = H // NCORES  # heads per core = 2
P = 128
CH = 512  # s-chunk width
BF16 = ml_dtypes.bfloat16


def build_nc(S):
    import concourse.bacc as bacc
    import concourse.mybir as mybir
    from concourse import tile

    f32 = mybir.dt.float32
    f32r = mybir.dt.float32r
    bf16 = mybir.dt.bfloat16
    ADD = mybir.AluOpType.add
    EXP = mybir.ActivationFunctionType.Exp

    NCHUNK = S // CH
    NT = S // P  # number of 128-row t-tiles
    ND = D // P  # 8 d-tiles

    nc = bacc.Bacc("TRN2", target_bir_lowering=False, debug=False)

    xt_d = nc.dram_tensor("xt", [D, S], bf16, kind="ExternalInput")
    wqk_d = nc.dram_tensor("wqk", [D, 2 * HPC * HD], bf16, kind="ExternalInput")
    wv_d = nc.dram_tensor("wv", [D, HPC * HD], bf16, kind="ExternalInput")
    bq_d = nc.dram_tensor("bq", [HPC * HD], f32, kind="ExternalInput")
    wo_d = nc.dram_tensor("wo", [HPC * HD, D], bf16, kind="ExternalInput")
    masks_d = nc.dram_tensor("masks", [P, 4, CH], bf16, kind="ExternalInput")
    yt_d = nc.dram_tensor("yt", [D, S], bf16, kind="ExternalOutput")

    # chunk-granularity DRAM views: partition-major [p, dtile, s]
    xt_v = xt_d[:].rearrange("(dt p) s -> p dt s", p=P)
    yt_v = yt_d[:].rearrange("(dt p) s -> p dt s", p=P)

    with tile.TileContext(nc) as tc:
        with (
            tc.tile_pool(name="consts", bufs=1) as consts,
            tc.tile_pool(name="xtp", bufs=3) as xtp,
            tc.tile_pool(name="ptp", bufs=4) as ptp,
            tc.tile_pool(name="nmp", bufs=4) as nmp,
            tc.tile_pool(name="otp", bufs=2) as otp,
            tc.tile_pool(name="rcp", bufs=4) as rcp,
            tc.tile_pool(name="ytp", bufs=2) as ytp,
            tc.tile_pool(name="scp", bufs=2, space="PSUM") as scp,
            tc.tile_pool(name="avp", bufs=2, space="PSUM") as avp,
            tc.tile_pool(name="qyp", bufs=2, space="PSUM") as qyp,
        ):
            # ---- persistent SBUF ----
            wqk_sb = consts.tile([P, ND, 2 * HPC * HD], bf16)
            wv_sb = consts.tile([P, ND, HPC * HD], bf16)
            bq_sb = consts.tile([P, 1], f32)
            wo_sb = consts.tile([HPC * HD, D], bf16)
            masks_sb = consts.tile([P, 4, CH], bf16)
            ones_sb = consts.tile([P, HD], f32r)
            qt_sb = consts.tile([P, S], bf16)  # Q^T: h0 parts 0-63, h1 64-127
            kt_sb = consts.tile([P, S], bf16)
            # V-hat per head: [t-part, NT tiles, 72] (cols 0-63 = V, 64 = ones)
            vhat = [
                consts.tile([P, NT, 72], bf16, tag=f"vhat{h}", name=f"vhat{h}")
                for h in range(HPC)
            ]

            nc.vector.memset(ones_sb[64:65, :], 1.0)
            for h in range(HPC):
                nc.vector.memset(vhat[h][:, :, 64:65], 1.0)

            xts = {}  # chunk j -> xt tile

            def load_x(j, split):
                xt_t = xtp.tile([P, ND, CH], bf16, tag="xt", name="xt_t")
                if split:  # per-dtile loads so first matmuls start early
                    for d in range(ND):
                        nc.sync.dma_start(
                            wqk_sb[:, d, :],
                            wqk_d[d * P : (d + 1) * P, :],
                        )
                        nc.sync.dma_start(
                            xt_t[:, d, :], xt_v[:, d, j * CH : (j + 1) * CH]
                        )
                else:
                    nc.sync.dma_start(xt_t[:], xt_v[:, :, j * CH : (j + 1) * CH])
                xts[j] = xt_t

            def emit_qkv_c(j, c):
                """Q (c=0) or K (c=1) projection for s-chunk j: one psum tile."""
                xt_t = xts[j]
                ps = qyp.tile([P, CH], f32, tag="qy", name=f"qkps{c}")
                for d in range(ND):
                    nc.tensor.matmul(
                        ps[:],
                        wqk_sb[:, d, c * P : (c + 1) * P],
                        xt_t[:, d, :],
                        start=(d == 0),
                        stop=(d == ND - 1),
                    )
                if c == 0:
                    nc.vector.tensor_scalar(
                        out=qt_sb[:, j * CH : (j + 1) * CH],
                        in0=ps[:],
                        scalar1=bq_sb[:, 0:1],
                        scalar2=None,
                        op0=ADD,
                    )
                else:
                    nc.vector.tensor_copy(kt_sb[:, j * CH : (j + 1) * CH], ps[:])

            def emit_v(j, sub):
                """V[t, d] for 128-t subtile `sub` of chunk j, both heads."""
                xt_t = xts[j]
                vps = qyp.tile([P, P], f32, tag="qy", name="vps")
                for d in range(ND):
                    nc.tensor.matmul(
                        vps[:],
                        xt_t[:, d, sub * P : (sub + 1) * P],
                        wv_sb[:, d, :],
                        start=(d == 0),
                        stop=(d == ND - 1),
                    )
                tt = 4 * j + sub
                for h in range(HPC):
                    nc.vector.tensor_copy(
                        vhat[h][:, tt, 0:64], vps[:, 64 * h : 64 * h + 64]
                    )

            def emit_proj(j, ot, e, tail=False):
                """output projection for chunk j, d-tile e."""
                yt_ps = qyp.tile([P, CH], f32, tag="qy", name="ytps")
                nc.tensor.matmul(
                    yt_ps[:],
                    wo_sb[:, e * P : (e + 1) * P],
                    ot[:],
                    start=True,
                    stop=True,
                )
                yt_st = yt_stage[j % 2]
                if tail and e % 2 == 1:  # tail: alternate copies DVE/ACT
                    nc.scalar.copy(yt_st[:, e, :], yt_ps[:])
                else:
                    nc.vector.tensor_copy(yt_st[:, e, :], yt_ps[:])
                if tail and e == ND // 2 - 1:  # tail: overlap DMA halves
                    nc.sync.dma_start(
                        yt_v[:, 0 : ND // 2, j * CH : (j + 1) * CH],
                        yt_st[:, 0 : ND // 2, :],
                    )
                elif tail and e == ND - 1:
                    nc.sync.dma_start(
                        yt_v[:, ND // 2 : ND, j * CH : (j + 1) * CH],
                        yt_st[:, ND // 2 : ND, :],
                    )
                elif not tail and e == ND - 1:
                    nc.sync.dma_start(yt_v[:, :, j * CH : (j + 1) * CH], yt_st[:])

            yt_stage = [
                ytp.tile([P, ND, CH], bf16, tag="yt", name=f"ytst{i}")
                for i in range(2)
            ]

            # ---- prologue: chunk 0 with interleaved weight loads ----
            load_x(0, split=True)
            nc.sync.dma_start(bq_sb[:], bq_d[:].rearrange("(i p) -> p i", p=P))
            emit_qkv_c(0, 1)
            nc.sync.dma_start(wv_sb[:], wv_d[:].rearrange("(dt p) c -> p dt c", p=P))
            emit_qkv_c(0, 0)
            nc.sync.dma_start(masks_sb[:], masks_d[:])
            nc.sync.dma_start(wo_sb[:], wo_d[:])
            load_x(1, split=False)
            for sub in range(4):
                emit_v(0, sub)
            emit_qkv_c(1, 0)
            emit_qkv_c(1, 1)
            for sub in range(4):
                emit_v(1, sub)

            # division state carried across chunks: (rcs, nms) per head
            carried = None  # (j_prev, rcs, nms)

            for j in range(NCHUNK):
                ntt = 4 * (j + 1)
                av = [
                    avp.tile([P, CH], f32, tag="av", name=f"av{h}")
                    for h in range(HPC)
                ]

                # ---- build filler list: PE work to interleave into groups ----
                fillers = []
                if carried is not None:
                    jp, rcs, nms = carried

                    def div_and_proj(jp=jp, rcs=rcs, nms=nms):
                        ot = otp.tile([P, CH], bf16, tag="ot", name="ot")
                        for h in range(HPC):
                            bc = qyp.tile([HD, CH], f32, tag="qy", name="bc")
                            nc.tensor.matmul(
                                bc[:],
                                ones_sb[64:65, 0:HD],
                                rcs[h][64:65, :],
                                start=True,
                                stop=True,
                            )
                            nc.vector.tensor_mul(
                                ot[64 * h : 64 * h + 64, :], nms[h][:], bc[:]
                            )
                        return ot

                    ot_box = []

                    def mk_div():
                        ot_box.append(div_and_proj())

                    fillers.append(mk_div)
                    for e in range(ND):
                        fillers.append(
                            lambda jp=jp, e=e: emit_proj(jp, ot_box[0], e)
                        )
                if j + 2 < NCHUNK:
                    jf = j + 2
                    fillers.append(lambda jf=jf: load_x(jf, split=False))
                    fillers.append(lambda jf=jf: emit_qkv_c(jf, 0))
                    fillers.append(lambda jf=jf: emit_qkv_c(jf, 1))
                    for sub in range(4):
                        fillers.append(lambda jf=jf, sub=sub: emit_v(jf, sub))

                # spread fillers over the group loop
                nfill = len(fillers)
                fill_every = max(1, ntt // max(nfill, 1))
                fill_i = 0

                def soff(tt):
                    o = (tt - 4 * j) * P if tt >= 4 * j else 0
                    return min(max(0, o), 3 * P)

                def flush(tt, sc):
                    """exp + mask + AV for t-tile tt (both heads)."""
                    o = soff(tt)
                    pt = ptp.tile([P, HPC, CH], bf16, tag="pt", name="pt")
                    sc_v = sc[:].rearrange("p (g c) -> p g c", c=CH)
                    nc.scalar.activation(
                        pt[:, :, o:], sc_v[:, :, o:], EXP, scale=0.125
                    )
                    if tt >= 4 * j:  # diagonal: one masked mul for both heads
                        k = tt - 4 * j
                        nc.vector.tensor_mul(
                            pt[:, :, o:],
                            pt[:, :, o:],
                            masks_sb[:, k : k + 1, o:].broadcast_to(
                                [P, HPC, CH - o]
                            ),
                        )
                    for h in range(HPC):
                        nc.tensor.matmul(
                            av[h][0:65, o:],
                            vhat[h][:, tt, 0:65],
                            pt[:, h, o:],
                            start=(tt == 0),
                            stop=(tt == ntt - 1),
                        )

                pending = None
                for tt in range(ntt):
                    o = soff(tt)
                    sc = scp.tile([P, HPC * CH], f32, tag="sc", name="sc")
                    for h in range(HPC):
                        nc.tensor.matmul(
                            sc[:, h * CH + o : (h + 1) * CH],
                            kt_sb[64 * h : 64 * h + 64, tt * P : (tt + 1) * P],
                            qt_sb[64 * h : 64 * h + 64, j * CH + o : (j + 1) * CH],
                            start=True,
                            stop=True,
                        )
                    # interleave filler PE work between score groups
                    if fill_i < nfill and (tt % fill_every == fill_every - 1):
                        fillers[fill_i]()
                        fill_i += 1
                    if pending is not None:
                        flush(*pending)
                    pending = (tt, sc)
                while fill_i < nfill:
                    fillers[fill_i]()
                    fill_i += 1
                if pending is not None:
                    flush(*pending)

                # ---- reciprocals + numerator copies (free the av tiles) ----
                rcs, nms = [], []
                for h in range(HPC):
                    rc = rcp.tile([P, CH], f32r, tag="rc", name="rc")
                    with nc.allow_low_precision("fp32r recip feeds fp22 matmul"):
                        nc.vector.reciprocal(rc[64:65, :], av[h][64:65, :])
                    nm = nmp.tile([HD, CH], f32, tag="nm", name="nm")
                    nc.vector.tensor_copy(nm[:], av[h][0:64, :])
                    rcs.append(rc)
                    nms.append(nm)
                carried = (j, rcs, nms)

            # ---- epilogue: division + projection for the last chunk ----
            jp, rcs, nms = carried
            ot = otp.tile([P, CH], bf16, tag="ot", name="ot")
            for h in range(HPC):
                bc = qyp.tile([HD, CH], f32, tag="qy", name="bc")
                nc.tensor.matmul(
                    bc[:],
                    ones_sb[64:65, 0:HD],
                    rcs[h][64:65, :],
                    start=True,
                    stop=True,
                )
                nc.vector.tensor_mul(ot[64 * h : 64 * h + 64, :], nms[h][:], bc[:])
            for e in range(ND):
                emit_proj(jp, ot, e, tail=True)

    return nc


@functools.lru_cache(maxsize=2)
def _get_nc(S):
    nc = build_nc(S)
    nc.compile()
    return nc


def make_in_maps(input, Wqkv, bqkv, Wo, S):
    """Host-side shard prep. input [1,S,D] (or [S,D]); returns per-core dicts."""
    x = np.asarray(input, dtype=np.float32).reshape(S, D)
    xt = np.ascontiguousarray(x.T.astype(BF16))
    Wqkv = np.asarray(Wqkv, dtype=np.float32)
    bqkv = np.asarray(bqkv, dtype=np.float32)
    Wo = np.asarray(Wo, dtype=np.float32)

    # causal masks for the 4 diagonal 128-blocks of a 512 chunk
    pp = np.arange(P)[:, None]
    ff = np.arange(CH)[None, :]
    masks = np.stack(
        [(ff >= pp + P * k).astype(BF16) for k in range(4)], axis=1
    )  # [128, 4, 512]
    masks = np.ascontiguousarray(masks)

    Wq, Wk, Wv = Wqkv[:, 0:D], Wqkv[:, D : 2 * D], Wqkv[:, 2 * D : 3 * D]
    bq = bqkv[0:D]

    in_maps = []
    for c in range(NCORES):
        hs = [c * HPC + i for i in range(HPC)]
        cols = lambda W: np.concatenate(
            [W[:, h * HD : (h + 1) * HD] for h in hs], axis=1
        )
        colsb = lambda b: np.concatenate(
            [b[h * HD : (h + 1) * HD] for h in hs], axis=0
        )
        wqk_l = np.ascontiguousarray(
            np.concatenate([cols(Wq), cols(Wk)], axis=1).astype(BF16)
        )
        wv_l = np.ascontiguousarray(cols(Wv).astype(BF16))
        bq_l = np.ascontiguousarray(colsb(bq).astype(np.float32))
        wo_l = np.ascontiguousarray(
            Wo[hs[0] * HD : hs[0] * HD + HPC * HD, :].astype(BF16)
        )
        in_maps.append(
            {
                "xt": xt,
                "wqk": wqk_l,
                "wv": wv_l,
                "bq": bq_l,
                "wo": wo_l,
                "masks": masks,
                "ones": np.ones((1, HD), dtype=np.float32),
            }
        )
    return in_maps


def kernel(input, Wqkv, bqkv, Wo, bo):
    from concourse.bass_utils import run_bass_kernel_spmd

    S = np.asarray(input).reshape(-1, D).shape[0]
    nc = _get_nc(S)
    in_maps = make_in_maps(input, Wqkv, bqkv, Wo, S)
    res = None
    last_exc = None
    for _attempt in range(3):  # transient NRT/device errors: retry
        try:
            res = run_bass_kernel_spmd(nc, in_maps, core_ids=list(range(NCORES)))
            break
        except Exception as e:  # noqa: BLE001
            last_exc = e
    if res is None:
        raise last_exc
    yt = res.results[0]["yt"].astype(np.float32)
    for r in res.results[1:]:
        yt += r["yt"].astype(np.float32)
    # fold the V bias through the output projection: y += bv @ Wo + bo
    bv = np.asarray(bqkv, dtype=np.float32)[2 * D : 3 * D]
    bo_eff = np.asarray(bo, dtype=np.float32) + bv @ np.asarray(
        Wo, dtype=np.float32
    )
    y = yt.T + bo_eff[None, :]
    return np.ascontiguousarray(y, dtype=np.float32).reshape(1, S, D)


# revision 10
# speedup vs baseline: 1.0216x; 1.0103x over previous
"""Multi-head causal self-attention (B=1, S=4096, D=1024, H=16) on 8 TRN2
NeuronCores, tensor-parallel over heads (2 heads per core).

v2: all-bf16 dataflow engineered to the TimelineSim cost model.
  - qkv^T = (X @ W)^T via matmul(lhsT=W_tile, rhs=X^T tile); K needs no bias
    (constant-in-t score shifts cancel in softmax), V's bias is folded into
    bo on the host (bo' = bo + bv @ Wo), so only Q gets a bias add.
  - V is produced directly in [t, d] layout by a second matmul pass with
    X^T tiles as the stationary operand (lhsT=xt[:,t-tile], rhs=Wv k-tile),
    eliminating all on-device transposes.
  - scores^T [t, s] per head via matmul(lhsT=K^T tile, rhs=Q^T chunk), full
    diagonal trim (bf16 matmuls have no free-dim>=256 constraint).
  - softmax without max-subtraction; exp on ACT with 1/8 scale folded in;
    pt in bf16 so the diagonal mask multiply runs in DVE 2x mode.
  - P^T @ V via matmul(lhsT=vhat[t,d]+ones column, rhs=pt) -> numerator rows
    0-63 and denominator on row 64 of PSUM.
  - denominator reciprocal broadcast back via a K=1 ones matmul; divide on
    DVE; y^T partial = Wo^T @ out^T; bf16 partials DMAed out once per chunk;
    host sums the 8 partials, adds bo', transposes.
  - one fused ~1MB DMA per chunk each way; proj/qkv/V matmuls of neighboring
    chunks are interleaved into the attention group loop as PE filler so the
    PE never idles behind the ACT-bound exp cadence.
"""

import sys

sys.path.insert(0, "/opt/trn_rl_repo")

import functools
import numpy as np
import ml_dtypes

D = 1024
H = 16
HD = 64
NCORES = 8
HPC = H // NCORES  # heads per core = 2
P = 128
CH = 512  # s-chunk width
BF16 = ml_dtypes.bfloat16


def build_nc(S):
    import concourse.bacc as bacc
    import concourse.mybir as mybir
    from concourse import tile

    f32 = mybir.dt.float32
    f32r = mybir.dt.float32r
    bf16 = mybir.dt.bfloat16
    ADD = mybir.AluOpType.add
    EXP = mybir.ActivationFunctionType.Exp

    NCHUNK = S // CH
    NT = S // P  # number of 128-row t-tiles
    ND = D // P  # 8 d-tiles

    nc = bacc.Bacc("TRN2", target_bir_lowering=False, debug=False)

    xt_d = nc.dram_tensor("xt", [D, S], bf16, kind="ExternalInput")
    wqk_d = nc.dram_tensor("wqk", [D, 2 * HPC * HD], bf16, kind="ExternalInput")
    wv_d = nc.dram_tensor("wv", [D, HPC * HD], bf16, kind="ExternalInput")
    bq_d = nc.dram_tensor("bq", [HPC * HD], f32, kind="ExternalInput")
    wo_d = nc.dram_tensor("wo", [HPC * HD, D], bf16, kind="ExternalInput")
    masks_d = nc.dram_tensor("masks", [P, 4, CH], bf16, kind="ExternalInput")
    yt_d = nc.dram_tensor("yt", [D, S], bf16, kind="ExternalOutput")

    # chunk-granularity DRAM views: partition-major [p, dtile, s]
    xt_v = xt_d[:].rearrange("(dt p) s -> p dt s", p=P)
    yt_v = yt_d[:].rearrange("(dt p) s -> p dt s", p=P)

    with tile.TileContext(nc) as tc:
        with (
            tc.tile_pool(name="consts", bufs=1) as consts,
            tc.tile_pool(name="xtp", bufs=3) as xtp,
            tc.tile_pool(name="ptp", bufs=4) as ptp,
            tc.tile_pool(name="nmp", bufs=4) as nmp,
            tc.tile_pool(name="otp", bufs=2) as otp,
            tc.tile_pool(name="rcp", bufs=4) as rcp,
            tc.tile_pool(name="ytp", bufs=2) as ytp,
            tc.tile_pool(name="scp", bufs=2, space="PSUM") as scp,
            tc.tile_pool(name="avp", bufs=2, space="PSUM") as avp,
            tc.tile_pool(name="qyp", bufs=2, space="PSUM") as qyp,
        ):
            # ---- persistent SBUF ----
            wqk_sb = consts.tile([P, ND, 2 * HPC * HD], bf16)
            wv_sb = consts.tile([P, ND, HPC * HD], bf16)
            bq_sb = consts.tile([P, 1], f32)
            wo_sb = consts.tile([HPC * HD, D], bf16)
            masks_sb = consts.tile([P, 4, CH], bf16)
            ones_sb = consts.tile([P, HD], f32r)
            qt_sb = consts.tile([P, S], bf16)  # Q^T: h0 parts 0-63, h1 64-127
            kt_sb = consts.tile([P, S], bf16)
            # V-hat per head: [t-part, NT tiles, 72] (cols 0-63 = V, 64 = ones)
            vhat = [
                consts.tile([P, NT, 72], bf16, tag=f"vhat{h}", name=f"vhat{h}")
                for h in range(HPC)
            ]

            nc.vector.memset(ones_sb[64:65, :], 1.0)
            for h in range(HPC):
                nc.vector.memset(vhat[h][:, :, 64:65], 1.0)

            xts = {}  # chunk j -> xt tile

            def load_x(j, split):
                xt_t = xtp.tile([P, ND, CH], bf16, tag="xt", name="xt_t")
                if split:  # per-dtile loads so first matmuls start early
                    for d in range(ND):
                        nc.sync.dma_start(
                            wqk_sb[:, d, :],
                            wqk_d[d * P : (d + 1) * P, :],
                        )
                        nc.sync.dma_start(
                            xt_t[:, d, :], xt_v[:, d, j * CH : (j + 1) * CH]
                        )
                else:
                    nc.sync.dma_start(xt_t[:], xt_v[:, :, j * CH : (j + 1) * CH])
                xts[j] = xt_t

            def emit_qkv_c(j, c):
                """Q (c=0) or K (c=1) projection for s-chunk j: one psum tile."""
                xt_t = xts[j]
                ps = qyp.tile([P, CH], f32, tag="qy", name=f"qkps{c}")
                for d in range(ND):
                    nc.tensor.matmul(
                        ps[:],
                        wqk_sb[:, d, c * P : (c + 1) * P],
                        xt_t[:, d, :],
                        start=(d == 0),
                        stop=(d == ND - 1),
                    )
                if c == 0:
                    nc.vector.tensor_scalar(
                        out=qt_sb[:, j * CH : (j + 1) * CH],
                        in0=ps[:],
                        scalar1=bq_sb[:, 0:1],
                        scalar2=None,
                        op0=ADD,
                    )
                else:
                    nc.vector.tensor_copy(kt_sb[:, j * CH : (j + 1) * CH], ps[:])

            def emit_v(j, sub):
                """V[t, d] for 128-t subtile `sub` of chunk j, both heads."""
                xt_t = xts[j]
                vps = qyp.tile([P, P], f32, tag="qy", name="vps")
                for d in range(ND):
                    nc.tensor.matmul(
                        vps[:],
                        xt_t[:, d, sub * P : (sub + 1) * P],
                        wv_sb[:, d, :],
                        start=(d == 0),
                        stop=(d == ND - 1),
                    )
                tt = 4 * j + sub
                for h in range(HPC):
                    nc.vector.tensor_copy(
                        vhat[h][:, tt, 0:64], vps[:, 64 * h : 64 * h + 64]
                    )

            def emit_proj(j, ot, e, tail=False):
                """output projection for chunk j, d-tile e."""
                yt_ps = qyp.tile([P, CH], f32, tag="qy", name="ytps")
                nc.tensor.matmul(
                    yt_ps[:],
                    wo_sb[:, e * P : (e + 1) * P],
                    ot[:],
                    start=True,
                    stop=True,
                )
                yt_st = yt_stage[j % 2]
                if tail and e % 2 == 1:  # tail: alternate copies DVE/ACT
                    nc.scalar.copy(yt_st[:, e, :], yt_ps[:])
                else:
                    nc.vector.tensor_copy(yt_st[:, e, :], yt_ps[:])
                if tail and e == ND // 2 - 1:  # tail: overlap DMA halves
                    nc.sync.dma_start(
                        yt_v[:, 0 : ND // 2, j * CH : (j + 1) * CH],
                        yt_st[:, 0 : ND // 2, :],
                    )
                elif tail and e == ND - 1:
                    nc.sync.dma_start(
                        yt_v[:, ND // 2 : ND, j * CH : (j + 1) * CH],
                        yt_st[:, ND // 2 : ND, :],
                    )
                elif not tail and e == ND - 1:
                    nc.sync.dma_start(yt_v[:, :, j * CH : (j + 1) * CH], yt_st[:])

            yt_stage = [
                ytp.tile([P, ND, CH], bf16, tag="yt", name=f"ytst{i}")
                for i in range(2)
            ]

            # ---- prologue: chunk 0 with interleaved weight loads ----
            load_x(0, split=True)
            nc.sync.dma_start(bq_sb[:], bq_d[:].rearrange("(i p) -> p i", p=P))
            nc.sync.dma_start(masks_sb[:], masks_d[:])
            emit_qkv_c(0, 1)
            nc.sync.dma_start(wv_sb[:], wv_d[:].rearrange("(dt p) c -> p dt c", p=P))
            emit_qkv_c(0, 0)
            load_x(1, split=False)
            nc.sync.dma_start(wo_sb[:], wo_d[:])
            for sub in range(4):
                emit_v(0, sub)
            emit_qkv_c(1, 1)
            emit_qkv_c(1, 0)
            for sub in range(4):
                emit_v(1, sub)

            # division state carried across chunks: (rcs, nms) per head
            carried = None  # (j_prev, rcs, nms)

            for j in range(NCHUNK):
                ntt = 4 * (j + 1)
                av = [
                    avp.tile([P, CH], f32, tag="av", name=f"av{h}")
                    for h in range(HPC)
                ]

                # next-next chunk's activations: DMA in flight ASAP
                if j + 2 < NCHUNK:
                    load_x(j + 2, split=False)

                # ---- build filler list: PE work to interleave into groups ----
                fillers = []
                if carried is not None:
                    jp, rcs, nms = carried

                    def div_and_proj(jp=jp, rcs=rcs, nms=nms):
                        ot = otp.tile([P, CH], bf16, tag="ot", name="ot")
                        for h in range(HPC):
                            bc = qyp.tile([HD, CH], f32, tag="qy", name="bc")
                            nc.tensor.matmul(
                                bc[:],
                                ones_sb[64:65, 0:HD],
                                rcs[h][64:65, :],
                                start=True,
                                stop=True,
                            )
                            nc.vector.tensor_mul(
                                ot[64 * h : 64 * h + 64, :], nms[h][:], bc[:]
                            )
                        return ot

                    ot_box = []

                    def mk_div():
                        ot_box.append(div_and_proj())

                    fillers.append(mk_div)
                    for e in range(ND):
                        fillers.append(
                            lambda jp=jp, e=e: emit_proj(jp, ot_box[0], e)
                        )
                if j + 2 < NCHUNK:
                    jf = j + 2
                    fillers.append(lambda jf=jf: emit_qkv_c(jf, 1))
                    fillers.append(lambda jf=jf: emit_qkv_c(jf, 0))
                    for sub in range(4):
                        fillers.append(lambda jf=jf, sub=sub: emit_v(jf, sub))

                # spread fillers over the group loop (skewed slightly late so
                # DMA-dependent fillers see their data landed)
                nfill = len(fillers)
                fill_pos = [((i + 1) * ntt) // (nfill + 1) for i in range(nfill)]
                fill_i = 0

                def soff(tt):
                    o = (tt - 4 * j) * P if tt >= 4 * j else 0
                    return min(max(0, o), 3 * P)

                def flush(tt, sc):
                    """exp + mask + AV for t-tile tt (both heads)."""
                    o = soff(tt)
                    pt = ptp.tile([P, HPC, CH], bf16, tag="pt", name="pt")
                    sc_v = sc[:].rearrange("p (g c) -> p g c", c=CH)
                    nc.scalar.activation(
                        pt[:, :, o:], sc_v[:, :, o:], EXP, scale=0.125
                    )
                    if tt >= 4 * j:  # diagonal: one masked mul for both heads
                        k = tt - 4 * j
                        nc.vector.tensor_mul(
                            pt[:, :, o:],
                            pt[:, :, o:],
                            masks_sb[:, k : k + 1, o:].broadcast_to(
                                [P, HPC, CH - o]
                            ),
                        )
                    for h in range(HPC):
                        nc.tensor.matmul(
                            av[h][0:65, o:],
                            vhat[h][:, tt, 0:65],
                            pt[:, h, o:],
                            start=(tt == 0),
                            stop=(tt == ntt - 1),
                        )

                pending = None
                for tt in range(ntt):
                    o = soff(tt)
                    sc = scp.tile([P, HPC * CH], f32, tag="sc", name="sc")
                    for h in range(HPC):
                        nc.tensor.matmul(
                            sc[:, h * CH + o : (h + 1) * CH],
                            kt_sb[64 * h : 64 * h + 64, tt * P : (tt + 1) * P],
                            qt_sb[64 * h : 64 * h + 64, j * CH + o : (j + 1) * CH],
                            start=True,
                            stop=True,
                        )
                    # interleave filler PE work between score groups
                    while fill_i < nfill and fill_pos[fill_i] <= tt:
                        fillers[fill_i]()
                        fill_i += 1
                    if pending is not None:
                        flush(*pending)
                    pending = (tt, sc)
                while fill_i < nfill:
                    fillers[fill_i]()
                    fill_i += 1
                if pending is not None:
                    flush(*pending)

                # ---- reciprocals + numerator copies (free the av tiles) ----
                rcs, nms = [], []
                for h in range(HPC):
                    rc = rcp.tile([P, CH], f32r, tag="rc", name="rc")
                    with nc.allow_low_precision("fp32r recip feeds fp22 matmul"):
                        nc.vector.reciprocal(rc[64:65, :], av[h][64:65, :])
                    nm = nmp.tile([HD, CH], f32, tag="nm", name="nm")
                    nc.vector.tensor_copy(nm[:], av[h][0:64, :])
                    rcs.append(rc)
                    nms.append(nm)
                carried = (j, rcs, nms)

            # ---- epilogue: division + projection for the last chunk ----
            jp, rcs, nms = carried
            ot = otp.tile([P, CH], bf16, tag="ot", name="ot")
            for h in range(HPC):
                bc = qyp.tile([HD, CH], f32, tag="qy", name="bc")
                nc.tensor.matmul(
                    bc[:],
                    ones_sb[64:65, 0:HD],
                    rcs[h][64:65, :],
                    start=True,
                    stop=True,
                )
                nc.vector.tensor_mul(ot[64 * h : 64 * h + 64, :], nms[h][:], bc[:])
            for e in range(ND):
                emit_proj(jp, ot, e, tail=True)

    return nc


@functools.lru_cache(maxsize=2)
def _get_nc(S):
    nc = build_nc(S)
    nc.compile()
    return nc


def make_in_maps(input, Wqkv, bqkv, Wo, S):
    """Host-side shard prep. input [1,S,D] (or [S,D]); returns per-core dicts."""
    x = np.asarray(input, dtype=np.float32).reshape(S, D)
    xt = np.ascontiguousarray(x.T.astype(BF16))
    Wqkv = np.asarray(Wqkv, dtype=np.float32)
    bqkv = np.asarray(bqkv, dtype=np.float32)
    Wo = np.asarray(Wo, dtype=np.float32)

    # causal masks for the 4 diagonal 128-blocks of a 512 chunk
    pp = np.arange(P)[:, None]
    ff = np.arange(CH)[None, :]
    masks = np.stack(
        [(ff >= pp + P * k).astype(BF16) for k in range(4)], axis=1
    )  # [128, 4, 512]
    masks = np.ascontiguousarray(masks)

    Wq, Wk, Wv = Wqkv[:, 0:D], Wqkv[:, D : 2 * D], Wqkv[:, 2 * D : 3 * D]
    bq = bqkv[0:D]

    in_maps = []
    for c in range(NCORES):
        hs = [c * HPC + i for i in range(HPC)]
        cols = lambda W: np.concatenate(
            [W[:, h * HD : (h + 1) * HD] for h in hs], axis=1
        )
        colsb = lambda b: np.concatenate(
            [b[h * HD : (h + 1) * HD] for h in hs], axis=0
        )
        wqk_l = np.ascontiguousarray(
            np.concatenate([cols(Wq), cols(Wk)], axis=1).astype(BF16)
        )
        wv_l = np.ascontiguousarray(cols(Wv).astype(BF16))
        bq_l = np.ascontiguousarray(colsb(bq).astype(np.float32))
        wo_l = np.ascontiguousarray(
            Wo[hs[0] * HD : hs[0] * HD + HPC * HD, :].astype(BF16)
        )
        in_maps.append(
            {
                "xt": xt,
                "wqk": wqk_l,
                "wv": wv_l,
                "bq": bq_l,
                "wo": wo_l,
                "masks": masks,
                "ones": np.ones((1, HD), dtype=np.float32),
            }
        )
    return in_maps


def kernel(input, Wqkv, bqkv, Wo, bo):
    from concourse.bass_utils import run_bass_kernel_spmd

    S = np.asarray(input).reshape(-1, D).shape[0]
    nc = _get_nc(S)
    in_maps = make_in_maps(input, Wqkv, bqkv, Wo, S)
    res = None
    last_exc = None
    for _attempt in range(3):  # transient NRT/device errors: retry
        try:
            res = run_bass_kernel_spmd(nc, in_maps, core_ids=list(range(NCORES)))
            break
        except Exception as e:  # noqa: BLE001
            last_exc = e
    if res is None:
        raise last_exc
    yt = res.results[0]["yt"].astype(np.float32)
    for r in res.results[1:]:
        yt += r["yt"].astype(np.float32)
    # fold the V bias through the output projection: y += bv @ Wo + bo
    bv = np.asarray(bqkv, dtype=np.float32)[2 * D : 3 * D]
    bo_eff = np.asarray(bo, dtype=np.float32) + bv @ np.asarray(
        Wo, dtype=np.float32
    )
    y = yt.T + bo_eff[None, :]
    return np.ascontiguousarray(y, dtype=np.float32).reshape(1, S, D)
